# revision 45
# baseline (speedup 1.0000x reference)
"""GNN message-passing kernel for Trainium2 (8 NeuronCores).

Computation (see problem reference):
    x  = h.reshape(N, V, D)
    y  = relu(A @ (x_v W1_v) + b1_v)   per view v     (A = segment-sum over edges)
    z  = relu(A @ (y_v W2)   + b2)     per view v
    out = z.reshape(N, V*H)

Key restructure: aggregation commutes with the per-view linear maps,
    A @ (x W) = (A @ x) W
so we aggregate raw features first (one gather per edge) and apply the small
dense weights to the aggregated 128-node chunks.

Mapping to hardware:
  - dst nodes are bin-packed into 8 cores x 50 chunks x 128 slots, balancing
    per-bin lo/hi edge counts so one uniform SPMD schedule fits every core.
  - per chunk, edges are gathered with dma_gather (int16 indices; tables are
    split at row 32768 into lo/hi windows to cover >32k rows). Gathers cycle
    over 4 SWDGE queues (num_swdge_queues=4): the Q7 descriptor generation
    that serializes a single queue (~6.6ns/row) parallelizes across queues,
    reaching ~3.5ns/row — near the DMA transfer rate.
  - features travel as bf16 (768B rows, rel-err ~2e-3, well inside the 2e-2
    gate); this halves HBM traffic vs fp32, halves the AllGathers, and lets
    the PE run at 1 cycle/row everywhere (fp32 matmul is 4 cycles/row).
  - scatter-into-nodes is a one-hot matmul: S[e, n] = (dst_rel[e] == n), built
    on-device with is_equal against an iota row; m_chunk = sum_t S_t^T @ X_t
    accumulated in PSUM.
  - after round 1, per-core y shards are AllGathered into one shared y table
    in THREE pieces at chunk boundaries [0,20)/[20,32)/[32,50). The first two
    pieces concatenate to exactly the 32768-row lo table, so each collective
    fires as soon as its chunks are computed and the transfers overlap the
    round-1 tail; round-2 lo gathers only wait on piece 2, hi on piece 3.
"""

import sys

if '/opt/trn_rl_repo' not in sys.path:
    sys.path.insert(0, '/opt/trn_rl_repo')

import numpy as np
import ml_dtypes

import concourse.bacc as bacc
import concourse.bass as bass
import concourse.mybir as mybir
import concourse.tile as tile
from concourse import bass_utils
from concourse.masks import make_identity

P = 128
N_NODES = 50000
N_EDGES = 400000
V = 3
D = 128
F = V * D            # 384 feature width (bf16 rows, 768B)
NCORES = 8
NCHUNK = 50          # chunks per core
SLOTS = NCHUNK * P   # 6400 slots per core
NFULL = NCORES * SLOTS  # 51200
SPLIT = 32768        # int16 gather-table row limit
NQ = 4               # SWDGE queues
# AllGather piece chunk boundaries; pieces 0-2 (chunks [0,32)) concatenate to
# exactly the 32768-row lo table, pieces 3-5 to the hi table. Fine pieces let
# each collective fire as soon as its chunks are computed, overlapping the
# round-1 tail.
PIECE_STARTS = [0, 8, 16, 24, 32, 41, 50]
NPIECE = len(PIECE_STARTS) - 1
LO_PIECES = PIECE_STARTS.index(32)   # pieces before this form the lo table
PIECE_DELAY = 5      # emit collectives this many chunks after their piece
                     # completes, so their y-write waits don't HOL-block the
                     # gpsimd queue between gathers

_BUILD_CACHE = {}


def _pack_bins(w_lo, w_hi, cap_lo, cap_hi):
    """Assign each node to a (core, chunk) bin: 400 bins x 128 slots,
    balancing lo/hi edge counts under the given caps. Returns slot[N]."""
    nbins = NCORES * NCHUNK
    n = len(w_lo)
    bin_lo = np.zeros(nbins, np.int64)
    bin_hi = np.zeros(nbins, np.int64)
    bin_cnt = np.zeros(nbins, np.int64)
    bin_members = [[] for _ in range(nbins)]
    order = np.argsort(-(w_lo + w_hi), kind='stable')
    wl = w_lo.astype(np.int64)
    wh = w_hi.astype(np.int64)
    for node in order:
        l, h = wl[node], wh[node]
        feas = (bin_cnt < P) & (bin_lo + l <= cap_lo) & (bin_hi + h <= cap_hi)
        if not feas.any():
            feas = bin_cnt < P
        load = np.maximum((bin_lo + l) / cap_lo, (bin_hi + h) / cap_hi)
        load = np.where(feas, load, np.inf)
        b = int(np.argmin(load))
        bin_members[b].append(node)
        bin_lo[b] += l
        bin_hi[b] += h
        bin_cnt[b] += 1
    slot = np.full(n, -1, np.int64)
    for b in range(nbins):
        for i, node in enumerate(bin_members[b]):
            slot[node] = b * P + i
    t_lo = int(-(-bin_lo.max() // P))
    t_hi = int(-(-bin_hi.max() // P))
    return slot, t_lo, t_hi


def _edge_tables(key, dst_slot, t_lo, t_hi):
    """Build per-core gather index + dst_rel arrays for one round.

    key: per-edge gather-table row (round-1: src id; round-2: y-table row of
    src). Returns idx_lo [8,50,t_lo*128] i16, idx_hi [8,50,t_hi*128] i16
    (pad 0), rel [8,50,(t_lo+t_hi)*128] f32 (pad -1); lo tiles then hi.
    """
    cap_l, cap_h = t_lo * P, t_hi * P
    e_bin = dst_slot // P
    e_rel = (dst_slot % P).astype(np.float64)
    is_hi = key >= SPLIT

    idx_lo = np.zeros((NCORES, NCHUNK, cap_l), np.int16)
    idx_hi = np.zeros((NCORES, NCHUNK, cap_h), np.int16)
    rel = np.full((NCORES, NCHUNK, cap_l + cap_h), -1.0, np.float32)

    order = np.lexsort((key, is_hi, e_bin))
    sb = e_bin[order]
    sh = is_hi[order]
    sk = key[order]
    sr = e_rel[order]
    grp = sb * 2 + sh
    new = np.ones(len(grp), bool)
    new[1:] = grp[1:] != grp[:-1]
    idxs = np.arange(len(grp))
    start = np.maximum.accumulate(np.where(new, idxs, 0))
    pos = idxs - start

    lo_m = ~sh
    b_lo, p_lo = sb[lo_m], pos[lo_m]
    assert p_lo.max(initial=0) < cap_l, "lo stream overflow; bump t_lo"
    idx_lo[b_lo // NCHUNK, b_lo % NCHUNK, p_lo] = sk[lo_m].astype(np.int16)
    rel[b_lo // NCHUNK, b_lo % NCHUNK, p_lo] = sr[lo_m]

    b_hi, p_hi = sb[sh], pos[sh]
    assert p_hi.max(initial=0) < cap_h, "hi stream overflow; bump t_hi"
    idx_hi[b_hi // NCHUNK, b_hi % NCHUNK, p_hi] = (sk[sh] - SPLIT).astype(np.int16)
    rel[b_hi // NCHUNK, b_hi % NCHUNK, cap_l + p_hi] = sr[sh]
    return idx_lo, idx_hi, rel


def _idx_layout(idx):
    """[NCORES, NCHUNK, cnt] -> [NCORES, 128, NCHUNK*cnt//16] int16 in the
    dma_gather wrapped layout (16-partition wrap, replicated x8)."""
    nc_, nch, cnt = idx.shape
    a = idx.reshape(nc_, nch, cnt // 16, 16)
    a = a.transpose(0, 3, 1, 2)
    a = a.reshape(nc_, 16, nch * (cnt // 16))
    return np.tile(a, (1, 8, 1)).copy()


def _rel_layout(rel):
    """[NCORES, NCHUNK, T*128] -> [NCORES, 128, NCHUNK*T] bf16; column
    (chunk*T + t) holds tile t's 128 dst_rel values."""
    nc_, nch, tot = rel.shape
    t = tot // P
    a = rel.reshape(nc_, nch, t, P)
    a = a.transpose(0, 3, 1, 2).reshape(nc_, P, nch * t)
    return np.ascontiguousarray(a).astype(ml_dtypes.bfloat16)


def _build(t_lo1, t_hi1, t_lo2, t_hi2, zero_bias=False):
    key = (t_lo1, t_hi1, t_lo2, t_hi2, zero_bias)
    if key in _BUILD_CACHE:
        return _BUILD_CACHE[key]

    nc = bacc.Bacc("TRN2", target_bir_lowering=False, debug=False,
                   num_devices=NCORES, num_swdge_queues=NQ)
    bf16 = mybir.dt.bfloat16
    f32 = mybir.dt.float32
    i16 = mybir.dt.int16

    h_pk = nc.dram_tensor("h_pk", [N_NODES, F], bf16, kind="ExternalInput")
    w1 = nc.dram_tensor("w1", [V, D, D], bf16, kind="ExternalInput")
    w2 = nc.dram_tensor("w2", [D, D], bf16, kind="ExternalInput")
    b1r = nc.dram_tensor("b1r", [P, F], f32, kind="ExternalInput")
    b2r = nc.dram_tensor("b2r", [P, F], f32, kind="ExternalInput")
    iota_in = nc.dram_tensor("iota", [P, P], bf16, kind="ExternalInput")
    idx1_lo = nc.dram_tensor("idx1_lo", [P, NCHUNK * t_lo1 * 8], i16, kind="ExternalInput")
    idx1_hi = nc.dram_tensor("idx1_hi", [P, NCHUNK * t_hi1 * 8], i16, kind="ExternalInput")
    idx2_lo = nc.dram_tensor("idx2_lo", [P, NCHUNK * t_lo2 * 8], i16, kind="ExternalInput")
    idx2_hi = nc.dram_tensor("idx2_hi", [P, NCHUNK * t_hi2 * 8], i16, kind="ExternalInput")
    rel1_in = nc.dram_tensor("rel1", [P, NCHUNK * (t_lo1 + t_hi1)], bf16, kind="ExternalInput")
    rel2_in = nc.dram_tensor("rel2", [P, NCHUNK * (t_lo2 + t_hi2)], bf16, kind="ExternalInput")
    z_out = nc.dram_tensor("z_out", [SLOTS, F], bf16, kind="ExternalOutput")

    piece_rows = [(PIECE_STARTS[k + 1] - PIECE_STARTS[k]) * P
                  for k in range(NPIECE)]
    y_sh = [nc.dram_tensor(f"y_sh{k}", [piece_rows[k], F], bf16,
                           kind="Internal")
            for k in range(NPIECE)]
    y_full = nc.dram_tensor("y_full", [NFULL, F], bf16, kind="Internal",
                            addr_space="Shared")
    piece_base = [0]
    for k in range(NPIECE):
        piece_base.append(piece_base[-1] + NCORES * piece_rows[k])
    assert piece_base[LO_PIECES] == SPLIT  # lo pieces form the lo table

    qi = [0]

    with tile.TileContext(nc) as tc:
        with (
            tc.tile_pool(name="const", bufs=1) as cpool,
            tc.tile_pool(name="glo", bufs=6) as glo_pool,
            tc.tile_pool(name="mlo", bufs=1) as mlo_pool,
            tc.tile_pool(name="ghi", bufs=4) as ghi_pool,
            tc.tile_pool(name="work", bufs=3) as work,
            tc.tile_pool(name="sel", bufs=4) as sel_pool,
            tc.tile_pool(name="ps_m", bufs=2, space="PSUM") as ps_m,
            tc.tile_pool(name="ps_y", bufs=2, space="PSUM") as ps_y,
            tc.tile_pool(name="ps_t", bufs=2, space="PSUM") as ps_t,
        ):
            # constants
            iota_t = cpool.tile([P, P], bf16)
            nc.sync.dma_start(iota_t[:], iota_in[:])
            ident = cpool.tile([P, P], bf16)
            make_identity(nc, ident[:])
            w1_t = cpool.tile([P, V * D], bf16)
            nc.sync.dma_start(
                w1_t[:].rearrange("d (v h) -> d v h", v=V),
                w1[:].rearrange("v d h -> d v h"),
            )
            w2_t = cpool.tile([P, D], bf16)
            nc.sync.dma_start(w2_t[:], w2[:])
            b1T_t = cpool.tile([P, F], f32)
            nc.sync.dma_start(b1T_t[:], b1r[:])
            b2_t = cpool.tile([P, F], f32)
            nc.sync.dma_start(b2_t[:], b2r[:])

            idx_tiles = {}
            for name, ten, tcount in (
                ("1lo", idx1_lo, t_lo1), ("1hi", idx1_hi, t_hi1),
                ("2lo", idx2_lo, t_lo2), ("2hi", idx2_hi, t_hi2),
            ):
                it = cpool.tile([P, NCHUNK * tcount * 8], i16, tag=f"idx{name}")
                nc.sync.dma_start(it[:], ten[:])
                idx_tiles[name] = it
            rel1_t = cpool.tile([P, NCHUNK * (t_lo1 + t_hi1)], bf16)
            nc.sync.dma_start(rel1_t[:], rel1_in[:])
            rel2_t = cpool.tile([P, NCHUNK * (t_lo2 + t_hi2)], bf16)
            nc.sync.dma_start(rel2_t[:], rel2_in[:])

            def gather(tag, pool, table, idx_t, c, t, dep=None):
                g = pool.tile([P, t * F], bf16, tag=tag)
                ins = nc.gpsimd.dma_gather(
                    out_ap=g[:].rearrange("p (t e) -> p t e", e=F),
                    in_ap=table,
                    idxs_ap=idx_t[:, c * t * 8:(c + 1) * t * 8],
                    num_idxs=t * P,
                    num_idxs_reg=t * P,
                    elem_size=F,
                    queue_num=qi[0] % NQ,
                )
                qi[0] += 1
                if dep is not None:
                    bass._add_dep_helper(
                        ins.ins, dep.ins, sync=True,
                        reason="round-2 gather after cross-core y barrier",
                    )
                return g

            def scatter_tiles(m_ps, c, ntile, t0, t1, g, goff, rel_t,
                              start, stop):
                """Accumulate one-hot scatter matmuls for tiles [t0, t1)."""
                for t in range(t0, t1):
                    s_t = sel_pool.tile([P, P], bf16, tag="s")
                    nc.vector.tensor_tensor(
                        out=s_t[:],
                        in0=rel_t[:, c * ntile + t: c * ntile + t + 1]
                            .to_broadcast([P, P]),
                        in1=iota_t[:],
                        op=mybir.AluOpType.is_equal,
                    )
                    nc.tensor.matmul(
                        out=m_ps[:], lhsT=s_t[:],
                        rhs=g[:, (t - goff) * F:(t - goff + 1) * F],
                        start=(t == t0 and start),
                        stop=(t == t1 - 1 and stop),
                    )

            def compute_chunk_r1(c, t_lo, t_hi, g_lo, g_hi, rel_t):
                ntile = t_lo + t_hi
                m_ps = ps_m.tile([P, F], f32, tag="m")
                scatter_tiles(m_ps, c, ntile, 0, t_lo, g_lo, 0, rel_t,
                              True, False)
                scatter_tiles(m_ps, c, ntile, t_lo, ntile, g_hi, t_lo, rel_t,
                              False, True)
                # round-1 transform: yw = relu(m @ W1 + b1) @ W2, written
                # row-major via   mT -> yT = W1^T mT -> relu -> yw = yT^T W2
                m_bf = work.tile([P, F], bf16, tag="m_bf")
                nc.scalar.copy(m_bf[:], m_ps[:])
                pt = ps_t.tile([P, F], bf16, tag="pt")
                for v in range(V):
                    nc.tensor.transpose(
                        pt[:, v * P:(v + 1) * P],
                        m_bf[:, v * P:(v + 1) * P], ident[:])
                mT = work.tile([P, F], bf16, tag="mT")
                nc.scalar.copy(mT[:], pt[:])
                yT_ps = ps_y.tile([P, F], f32, tag="yT")
                for v in range(V):
                    nc.tensor.matmul(
                        out=yT_ps[:, v * P:(v + 1) * P],
                        lhsT=w1_t[:, v * P:(v + 1) * P],
                        rhs=mT[:, v * P:(v + 1) * P],
                        start=True, stop=True,
                    )
                yT_bf = work.tile([P, F], bf16, tag="yT_bf")
                if zero_bias:
                    nc.scalar.activation(
                        yT_bf[:], yT_ps[:], mybir.ActivationFunctionType.Relu)
                else:
                    yT_f = work.tile([P, F], f32, tag="yT_f")
                    nc.vector.tensor_add(yT_f[:], yT_ps[:], b1T_t[:])
                    nc.scalar.activation(
                        yT_bf[:], yT_f[:], mybir.ActivationFunctionType.Relu)
                yw_ps = ps_t.tile([P, F], f32, tag="yw")
                for v in range(V):
                    nc.tensor.matmul(
                        out=yw_ps[:, v * P:(v + 1) * P],
                        lhsT=yT_bf[:, v * P:(v + 1) * P],
                        rhs=w2_t[:], start=True, stop=True,
                    )
                yw_bf = work.tile([P, F], bf16, tag="yw_bf")
                nc.scalar.copy(yw_bf[:], yw_ps[:])
                k = 0
                while c >= PIECE_STARTS[k + 1]:
                    k += 1
                r0 = (c - PIECE_STARTS[k]) * P
                nc.sync.dma_start(y_sh[k][r0:r0 + P, :], yw_bf[:])

            def emit_piece(k):
                return nc.gpsimd.collective_compute(
                    "AllGather", mybir.AluOpType.bypass,
                    replica_groups=[list(range(NCORES))],
                    ins=[y_sh[k][:]],
                    outs=[y_full[piece_base[k]:piece_base[k + 1], :]],
                )

            # ---- round 1 (pieces fire a few chunks after their boundary
            # so their y-write waits don't stall the gather stream, while
            # the transfers still overlap the round-1 tail) ----
            emitted = 0
            for c in range(NCHUNK):
                g_lo = gather("glo", glo_pool, h_pk[:SPLIT],
                              idx_tiles["1lo"], c, t_lo1)
                g_hi = gather("ghi", ghi_pool, h_pk[SPLIT:],
                              idx_tiles["1hi"], c, t_hi1)
                compute_chunk_r1(c, t_lo1, t_hi1, g_lo, g_hi, rel1_t)
                while (emitted < NPIECE - 1
                       and c + 1 >= PIECE_STARTS[emitted + 1] + PIECE_DELAY):
                    emit_piece(emitted)
                    emitted += 1
            last_cc = None
            while emitted < NPIECE:
                last_cc = emit_piece(emitted)
                emitted += 1

            # ---- round 2, phase A: aggregate every chunk's lo tiles into
            # SBUF staging. Starts as soon as the lo table (pieces 0-2) has
            # landed, overlapping the remaining hi-table collectives. ----
            # Gate phase A behind the final collective: the hi-table pieces
            # transfer ~4x faster without gather contention, which more than
            # pays back the lost overlap.
            ntile2 = t_lo2 + t_hi2
            m_lo = {}
            for c in range(NCHUNK):
                g_lo = gather("glo", glo_pool, y_full[:SPLIT],
                              idx_tiles["2lo"], c, t_lo2,
                              dep=last_cc if c < NQ else None)
                mp = ps_m.tile([P, F], f32, tag="m")
                scatter_tiles(mp, c, ntile2, 0, t_lo2, g_lo, 0, rel2_t,
                              True, True)
                ml = mlo_pool.tile([P, F], bf16, tag=f"mlo{c}")
                nc.scalar.copy(ml[:], mp[:])
                m_lo[c] = ml

            # ---- round 2, phase B: hi tiles + combine + relu ----
            for c in range(NCHUNK):
                g_hi = gather("ghi", ghi_pool, y_full[SPLIT:],
                              idx_tiles["2hi"], c, t_hi2)
                mp = ps_m.tile([P, F], f32, tag="m")
                scatter_tiles(mp, c, ntile2, t_lo2, ntile2, g_hi, t_lo2,
                              rel2_t, True, True)
                z_f = work.tile([P, F], f32, tag="z_f")
                nc.vector.tensor_add(z_f[:], mp[:], m_lo.pop(c)[:])
                if not zero_bias:
                    nc.vector.tensor_add(z_f[:], z_f[:], b2_t[:])
                z_sb = work.tile([P, F], bf16, tag="z_sb")
                nc.scalar.activation(
                    z_sb[:], z_f[:], mybir.ActivationFunctionType.Relu)
                nc.sync.dma_start(z_out[c * P:(c + 1) * P, :], z_sb[:])

    nc.compile()
    _BUILD_CACHE[key] = nc
    return nc


def prep_inputs(h, src, dst, W1, b1, W2, b2):
    h = np.asarray(h, np.float32)
    src = np.asarray(src).astype(np.int64)
    dst = np.asarray(dst).astype(np.int64)
    W1 = np.asarray(W1, np.float32)
    b1 = np.asarray(b1, np.float32)
    W2 = np.asarray(W2, np.float32)
    b2 = np.asarray(b2, np.float32)

    # ---- host prep: index tables (integer metadata only) ----
    # round 1: gather key = src id (table = h split at 32768)
    lo1 = src < SPLIT
    w_lo1 = np.bincount(dst[lo1], minlength=N_NODES)
    w_hi1 = np.bincount(dst[~lo1], minlength=N_NODES)
    p1, t_lo1, t_hi1 = _pack_bins(w_lo1, w_hi1, 6 * P, 3 * P)
    t_lo1, t_hi1 = max(t_lo1, 6), max(t_hi1, 3)

    # round 2: gather key = global y-table row of src. y_full is the concat
    # of the AllGathered pieces (chunk ranges PIECE_STARTS of each core);
    # pieces 0-2 form rows [0, 32768) = the lo table.
    core1 = p1 // SLOTS
    row1 = p1 % SLOTS
    piece_rows = [(PIECE_STARTS[k + 1] - PIECE_STARTS[k]) * P
                  for k in range(NPIECE)]
    piece_base = [0]
    for k in range(NPIECE):
        piece_base.append(piece_base[-1] + NCORES * piece_rows[k])
    starts = np.array([PIECE_STARTS[k] * P for k in range(NPIECE)])
    pk = np.searchsorted(starts[1:], row1, side='right')
    ytab = (np.array(piece_base[:NPIECE])[pk]
            + core1 * np.array(piece_rows)[pk] + (row1 - starts[pk]))
    key2 = ytab[src]
    lo2 = key2 < SPLIT
    w_lo2 = np.bincount(dst[lo2], minlength=N_NODES)
    w_hi2 = np.bincount(dst[~lo2], minlength=N_NODES)
    p2, t_lo2, t_hi2 = _pack_bins(w_lo2, w_hi2, 6 * P, 3 * P)
    t_lo2, t_hi2 = max(t_lo2, 6), max(t_hi2, 3)

    i1l, i1h, r1 = _edge_tables(src, p1[dst], t_lo1, t_hi1)
    i2l, i2h, r2 = _edge_tables(key2, p2[dst], t_lo2, t_hi2)

    i1l, i1h = _idx_layout(i1l), _idx_layout(i1h)
    i2l, i2h = _idx_layout(i2l), _idx_layout(i2h)
    r1, r2 = _rel_layout(r1), _rel_layout(r2)

    h_pk = h.astype(ml_dtypes.bfloat16)
    w1_bf = W1.astype(ml_dtypes.bfloat16)
    w2_bf = W2.astype(ml_dtypes.bfloat16)
    b2_flat = np.tile(b2, V)
    zero_bias = not (b1.any() or b2_flat.any())
    # b1 in yT layout: [h, v*128+n] = b1[v, h]
    b1_rep = np.ascontiguousarray(np.repeat(b1.T, P, axis=1)).astype(np.float32)
    b2_rep = np.broadcast_to(b2_flat, (P, F)).copy()
    iota = np.broadcast_to(np.arange(P, dtype=np.float32), (P, P)).astype(
        ml_dtypes.bfloat16).copy()

    in_maps = []
    for c in range(NCORES):
        in_maps.append({
            "h_pk": h_pk, "w1": w1_bf, "w2": w2_bf,
            "b1r": b1_rep, "b2r": b2_rep, "iota": iota,
            "idx1_lo": i1l[c], "idx1_hi": i1h[c],
            "idx2_lo": i2l[c], "idx2_hi": i2h[c],
            "rel1": r1[c], "rel2": r2[c],
        })

    return {
        "in_maps": in_maps,
        "tvals": (t_lo1, t_hi1, t_lo2, t_hi2),
        "zero_bias": zero_bias,
        "p2": p2,
    }


TRACE = False
LAST_RESULT = None


def kernel(h, src, dst, W1, b1, W2, b2):
    global LAST_RESULT
    prep = prep_inputs(h, src, dst, W1, b1, W2, b2)
    nc = _build(*prep["tvals"], zero_bias=prep["zero_bias"])
    res = bass_utils.run_bass_kernel_spmd(
        nc, prep["in_maps"], core_ids=list(range(NCORES)),
        trace=TRACE,
    )
    LAST_RESULT = res
    z_full = np.concatenate([res.results[c]["z_out"] for c in range(NCORES)],
                            axis=0)
    return z_full[prep["p2"]].astype(np.float32)
